# revision 37
# baseline (speedup 1.0000x reference)
"""MultiHeadAttention (cross-attention, B=32 N=512 L=1024 D=512 H=8) on 8 TRN2 cores.

Strategy: data parallelism (4 batches/core) + host-side sparsity compaction.

Host prep (inside kernel(), plain numpy):
  - per batch, gather the unmasked K/V positions (~50% of L=1024), pad to
    L_C=640 (5*128); padded slots get zero K/V rows and a -87 exp bias so they
    vanish from the softmax exactly like reference's -inf masking
  - x_q / x_kv pre-transposed AND pre-split into fp8e4m3 hi+lo pairs (hi =
    fp8(x), lo = fp8(x - hi)); Wq/Wk/Wv likewise; Wo in bf16
Device per-core dataflow:
  Q/K/V projections as fp8 DoubleRow matmuls, 3 term chains
  (x_hi*W_hi + x_hi*W_lo + x_lo*W_hi; dropped lo*lo term is ~0.07%) -> 0.75x
  precision-preserving at 4x DoubleRow rate = 3x faster than f32r.
  scores S^T[l,n] per head-pair packed via tile_position (K=64 row groups),
  both heads' scores in one [128,1024] f32 PSUM tile (f32r matmuls, exact)
  exp on ACT with per-partition bias (pad masking, -4 shift for fp8 range),
  fp8e4m3 output, l-chunk-paired tiles
  stage2 emits O[n,c] PER HEAD (M=n): stationary = P^T l-chunk pairs
  (DoubleRow k-tiles), moving = V|ones fp8; TWO chains (V_hi, V_lo residual)
  keep V at ~fp16 precision; out free dim is only 66 -> 33 cycles/matmul.
  denominator rides along as a ones column -> per-PARTITION reciprocal +
  broadcast multiply (cheap), PE-transpose (bf16) back to O^T[d,n] for o_proj
  o_proj in bf16, + bias, DMA out.
Emission is software-pipelined: prep (DMAs + QKV projections) of batch b+1 is
interleaved into the attention phase of batch b.
"""
import sys

sys.path.insert(0, "/opt/trn_rl_repo")
import numpy as np

B, N, L, D, H, C = 32, 512, 1024, 512, 8, 64
NCORES = 8
BLOC = B // NCORES  # 4 batches per core
SCALE = C ** -0.5
MASK_NEG = -87.0
EXP_SHIFT = -4.0  # keeps unnormalized exp inside fp8e4m3 range (max scaled
                  # score ~7.7 -> e^3.7 ~= 40 << 240); cancels in softmax
W_PRESCALE = 32.0  # lifts W out of fp8 subnormal range; q,k,v scaled by 32
P = 128
NDC = D // P   # 4 d/e chunks
NNC = N // P   # 4 n chunks
LC_SPARSE = 640

_CACHE = {}


def _nspans(l_c):
    # PSUM bank is 512 f32 -> split K-proj output into spans <=512
    if l_c == 640:
        return [(0, 384), (384, 640)]
    return [(s, min(s + 512, l_c)) for s in range(0, l_c, 512)]


def _build_nc(l_chunks):
    import concourse.bacc as bacc
    import concourse.tile as tile
    from concourse import mybir

    f32 = mybir.dt.float32
    f32r = mybir.dt.float32r
    bf16 = mybir.dt.bfloat16
    fp8 = mybir.dt.float8e4
    DR = mybir.MatmulPerfMode.DoubleRow
    EXP = mybir.ActivationFunctionType.Exp
    MUL = mybir.AluOpType.mult
    SUB = mybir.AluOpType.subtract
    L_C = l_chunks * P
    SC_EXP = SCALE / (W_PRESCALE * W_PRESCALE)
    # fp8 term chains: (x_hi*W_hi), (x_hi*W_lo), (x_lo*W_hi)
    TERMS = ((0, 0), (0, 1), (1, 0))

    nc = bacc.Bacc()
    xq8_d = nc.declare_dram_parameter("xq8", [BLOC, P, 2, NDC, N], fp8, isOutput=False)
    xk8_d = nc.declare_dram_parameter("xk8", [BLOC, P, 2, NDC, L_C], fp8, isOutput=False)
    rpbT_d = nc.declare_dram_parameter("rpbT", [BLOC, P, NDC, L_C], bf16, isOutput=False)
    mb_d = nc.declare_dram_parameter("mbias", [BLOC, L_C], f32, isOutput=False)
    Wq8_d = nc.declare_dram_parameter("Wq8", [P, 2, NDC, D], fp8, isOutput=False)
    Wk8_d = nc.declare_dram_parameter("Wk8", [P, 2, NDC, D], fp8, isOutput=False)
    Wv8_d = nc.declare_dram_parameter("Wv8", [P, 2, NDC, D], fp8, isOutput=False)
    Wo16_d = nc.declare_dram_parameter("Wo16", [D, D], bf16, isOutput=False)
    id_d = nc.declare_dram_parameter("ident", [P, P], bf16, isOutput=False)
    bo = nc.declare_dram_parameter("bo", [1, D], f32, isOutput=False)
    out = nc.declare_dram_parameter("out", [BLOC, N, D], f32, isOutput=True)

    with tile.TileContext(nc) as tc:
        with (
            tc.tile_pool(name="consts", bufs=1) as consts,
            tc.tile_pool(name="xin", bufs=2) as xin_pool,
            tc.tile_pool(name="qkt", bufs=2) as qkt_pool,
            tc.tile_pool(name="vp", bufs=2) as vp_pool,
            tc.tile_pool(name="pt", bufs=2) as pt_pool,
            tc.tile_pool(name="ot", bufs=2) as ot_pool,
            tc.tile_pool(name="outst", bufs=3) as outst_pool,
            tc.tile_pool(name="small", bufs=2) as small,
            tc.tile_pool(name="ps_sc", bufs=2, space="PSUM") as ps_sc,
            tc.tile_pool(name="ps_mm", bufs=2, space="PSUM") as ps_mm,
            tc.tile_pool(name="ps_po", bufs=2, space="PSUM") as ps_po,
        ):
            state = {}

            # ---- one-time setup ----
            warm = consts.tile([P, 1], f32, tag="warm")
            nc.vector.memset(warm, 0.0)
            nc.scalar.activation(out=warm, in_=warm, func=EXP, scale=1.0)

            ones8 = consts.tile([P, H], f32, tag="ones8")
            nc.vector.memset(ones8, 1.0)

            wsb = {}

            # Wq/xq hi halves first so Q-proj's hi*hi DR insts start after
            # ~2KB of DMA
            wq = consts.tile([P, 2, NDC, D], fp8, tag="wq", name="wq")
            xq0 = xin_pool.tile([P, 2, NDC, N], fp8, tag="xq", name="xq")
            for hl in range(2):
                nc.sync.dma_start(out=wq[:, hl], in_=Wq8_d[:, hl])
                nc.sync.dma_start(out=xq0[:, hl], in_=xq8_d[0, :, hl])
            state[(0, "xq")] = xq0
            wsb["q"] = wq

            def load_w8(key, Wd):
                wt = consts.tile([P, 2, NDC, D], fp8, tag=f"w{key}",
                                 name=f"w{key}")
                nc.sync.dma_start(out=wt, in_=Wd[:])
                wsb[key] = wt

            def load_wo():
                for k in range(NDC):
                    wt = consts.tile([P, D], bf16, tag=f"wo{k}", name=f"wo{k}")
                    nc.sync.dma_start(out=wt, in_=Wo16_d[k * P:(k + 1) * P, :])
                    wsb[("o", k)] = wt
                idt = consts.tile([P, P], bf16, tag="idt")
                nc.sync.dma_start(out=idt, in_=id_d[:])
                wsb["idt"] = idt
                bo_row = consts.tile([1, D], f32, tag="bo_row")
                nc.sync.dma_start(out=bo_row, in_=bo[:])
                bo_bc = consts.tile([P, D], f32, tag="bo_bc")
                nc.gpsimd.partition_broadcast(bo_bc, bo_row[0:1, :], channels=P)
                wsb["bo"] = bo_bc

            def proj_dr(pacc, ws, xs, stat_sel, mov_sel, x_stat):
                """Emit the 6-matmul fp8 DoubleRow projection group."""
                nt = len(TERMS)
                for ti, (xi, wi) in enumerate(TERMS):
                    for u in range(NDC // 2):
                        ku = slice(2 * u, 2 * u + 2)
                        if x_stat:
                            lhsT = xs[:, xi, ku, stat_sel]
                            rhs = ws[:, wi, ku, mov_sel]
                        else:
                            lhsT = ws[:, wi, ku, stat_sel]
                            rhs = xs[:, xi, ku, mov_sel]
                        nc.tensor.matmul(
                            pacc, lhsT, rhs,
                            start=(ti == 0 and u == 0),
                            stop=(ti == nt - 1 and u == NDC // 2 - 1),
                            perf_mode=DR)

            # ---- pipelined prep units (finer grain than the old slices so
            # kT lands earlier relative to its consumers) ----
            def qproj_js(b, js):
                xq = state[(b, "xq")]
                qT = state.setdefault((b, "qT"), {})
                for j in js:
                    pq = ps_mm.tile([P, N], f32, tag="mm", name="pq")
                    proj_dr(pq, wsb["q"], xq,
                            slice(j * P, (j + 1) * P), slice(0, N), False)
                    t = qkt_pool.tile([P, N], f32r, tag=f"qT{j}", name=f"qT{j}")
                    nc.vector.tensor_copy(t, pq)
                    qT[j] = t

            def kproj_js(b, js):
                xk = state[(b, "xk")]
                rpbT = state[(b, "rpbT")]
                kT = state.setdefault((b, "kT"), {})
                for j in js:
                    t = qkt_pool.tile([P, L_C], f32r, tag=f"kT{j}", name=f"kT{j}")
                    for (n0, n1) in _nspans(L_C):
                        pk = ps_mm.tile([P, N], f32, tag="mm", name="pk")
                        proj_dr(pk[:, 0:n1 - n0], wsb["k"], xk,
                                slice(j * P, (j + 1) * P), slice(n0, n1), False)
                        nc.vector.tensor_add(
                            t[:, n0:n1], pk[:, 0:n1 - n0], rpbT[:, j, n0:n1])
                    kT[j] = t

            def vproj_is(b, iis):
                xk = state[(b, "xk")]
                vP = state.setdefault((b, "vP"), {})
                for i in iis:
                    t = vp_pool.tile([P, H, C + 2], bf16, tag=f"vp{i}",
                                     name=f"vp{i}")
                    pv = ps_mm.tile([P, N], f32, tag="mm", name="pv")
                    proj_dr(pv, wsb["v"], xk,
                            slice(i * P, (i + 1) * P), slice(0, D), True)
                    nc.vector.tensor_copy(
                        t[:, :, 0:C], pv.rearrange("p (h c) -> p h c", h=H))
                    nc.vector.tensor_copy(t[:, :, C:C + 1], ones8[:, :, None])
                    nc.vector.memset(t[:, :, C + 1:C + 2], 0.0)
                    vP[i] = t

            def dma_qside(b):
                mb = small.tile([P, l_chunks], f32, tag="mbias")
                nc.sync.dma_start(
                    out=mb, in_=mb_d[b, :].rearrange("(i p) -> p i", p=P))
                state[(b, "mbias")] = mb
                if (b, "xq") not in state:
                    t = xin_pool.tile([P, 2, NDC, N], fp8, tag="xq", name="xq")
                    nc.sync.dma_start(out=t, in_=xq8_d[b])
                    state[(b, "xq")] = t

            def dma_kside(b, split_rpb=False):
                t = xin_pool.tile([P, 2, NDC, L_C], fp8, tag="xk", name="xk")
                nc.sync.dma_start(out=t, in_=xk8_d[b])
                state[(b, "xk")] = t
                r = xin_pool.tile([P, NDC, L_C], bf16, tag="rpbT", name="rpbT")
                if split_rpb:
                    nc.sync.dma_start(out=r[:, 0:1], in_=rpbT_d[b, :, 0:1])
                    nc.sync.dma_start(out=r[:, 1:], in_=rpbT_d[b, :, 1:])
                else:
                    nc.sync.dma_start(out=r, in_=rpbT_d[b])
                state[(b, "rpbT")] = r

            def prep_slice(b, sl):
                if sl == 0:
                    dma_qside(b)
                    dma_kside(b)
                    qproj_js(b, range(NDC))
                elif sl == 1:
                    kproj_js(b, (0, 1))
                elif sl == 2:
                    kproj_js(b, (2, 3))
                    state.pop((b, "rpbT"))
                elif sl == 3:
                    vproj_is(b, range(l_chunks))
                    state.pop((b, "xk"))
                    state.pop((b, "xq"))

            def attn_scores(b, j):
                mb = state[(b, "mbias")]
                qT, kT = state[(b, "qT")], state[(b, "kT")]
                ptiles = []
                for i in range(l_chunks):
                    pt = pt_pool.tile([P, 2 * N], bf16, tag=f"pt{i}",
                                      name=f"pt{i}")
                    pss = ps_sc.tile([P, 2 * N], f32, tag="sc", name="pss")
                    for half in range(2):
                        lo = 64 * half
                        nc.tensor.matmul(
                            pss[:, half * N:(half + 1) * N],
                            kT[j][lo:lo + 64, i * P:(i + 1) * P],
                            qT[j][lo:lo + 64, :], start=True, stop=True,
                            tile_position=(lo, 0))
                    nc.scalar.activation(out=pt, in_=pss, func=EXP,
                                         bias=mb[:, i:i + 1], scale=SC_EXP)
                    ptiles.append(pt)
                return ptiles

            def attn_stage2(b, j, ptiles):
                vP = state[(b, "vP")]
                oT = state[(b, "oT")]
                # stage2: out O[n, c] per head (M = n); stationary = P^T
                # chunks (bf16), moving = V|1 bf16, free dim 66.
                po = {}
                for he in range(2):
                    po[he] = ps_po.tile([P, NNC, C + 2], f32, tag="po", name="po")
                # PSUM zero-regions are whole 2KB banks: start=True may only
                # be issued ONCE per po tile (it flags the full bank pending-
                # zero); later first-writes to still-pending bytes overwrite.
                for i in range(l_chunks):
                    for he in range(2):
                        for m in range(NNC):
                            sel = slice(he * N + m * P, he * N + (m + 1) * P)
                            nc.tensor.matmul(
                                po[he][:, m, :], ptiles[i][:, sel],
                                vP[i][:, 2 * j + he, :],
                                start=(i == 0 and m == 0),
                                stop=(i == l_chunks - 1 and m == NNC - 1),
                                skip_group_check=True)
                oN = ot_pool.tile([P, NNC, 2, C], bf16, tag="oN", name="oN")
                for he in range(2):
                    rc = small.tile([P, NNC], f32, tag="rc", name="rc")
                    nc.vector.reciprocal(rc, po[he][:, :, C:C + 1].squeeze(2))
                    nc.vector.tensor_mul(
                        oN[:, :, he, :], po[he][:, :, 0:C],
                        rc.unsqueeze(2).broadcast_to([P, NNC, C]))
                return oN

            def transposes(b, j, oN, fuse_oproj=False):
                oT = state[(b, "oT")]
                for m in range(NNC):
                    ptr = ps_mm.tile([P, P], bf16, tag="mm", name="ptr")
                    nc.tensor.transpose(ptr, oN[:, m, :, :], wsb["idt"])
                    nc.vector.tensor_copy(oT[j][:, m * P:(m + 1) * P], ptr)
                    if fuse_oproj:
                        oproj_chunk(b, m)

            def oproj_chunk(b, m):
                oT = state[(b, "oT")]
                pf = ps_mm.tile([P, N], f32, tag="mm", name="pf")
                for k in range(NDC):
                    nc.tensor.matmul(pf, oT[k][:, m * P:(m + 1) * P],
                                     wsb[("o", k)], start=(k == 0),
                                     stop=(k == NDC - 1))
                to = outst_pool.tile([P, D], f32, tag="outst", name="to")
                nc.vector.tensor_add(to, pf, wsb["bo"])
                nc.sync.dma_start(out=out[b, m * P:(m + 1) * P, :], in_=to)

            # ---- main pipeline ----
            load_w8("k", Wk8_d)
            dma_kside(0, split_rpb=True)
            dma_qside(0)
            qproj_js(0, range(NDC))
            kproj_js(0, (0, 1))
            load_w8("v", Wv8_d)
            kproj_js(0, (2, 3))
            pts = {}
            pts[(0, 0)] = attn_scores(0, 0)  # first exps start before V-proj
            state.pop((0, "rpbT"))
            vproj_is(0, range(l_chunks))
            state.pop((0, "xk"))
            state.pop((0, "xq"))
            load_wo()
            for b in range(BLOC):
                state[(b, "oT")] = [
                    ot_pool.tile([P, N], bf16, tag=f"oT{j}", name=f"oT{j}")
                    for j in range(NDC)]
            pend_tr = None  # (b, j, oN) awaiting transposes
            for b in range(BLOC):
                for j in range(NDC):
                    if (b, j) not in pts:
                        pts[(b, j)] = attn_scores(b, j)
                    # lookahead: next phase's scores (cross-batch too) keep
                    # ACT fed through stage2/transpose/o_proj clumps; depth 2
                    # near the end so the exp stream finishes as early as
                    # possible (shrinks the tail)
                    t = b * NDC + j
                    for la in range(t + 1, min(t + 2, BLOC * NDC)):
                        lb, lj = divmod(la, NDC)
                        if (lb, lj) not in pts:
                            pts[(lb, lj)] = attn_scores(lb, lj)
                    if pend_tr is not None:
                        # deferred transposes: DVE had a full window to
                        # finish the previous phase's normalize
                        transposes(*pend_tr)
                        pend_tr = None
                    if b > 0:
                        # previous batch's o_proj interleaves into this
                        # batch's attention phases (fills PE during exp)
                        oproj_chunk(b - 1, j)
                    oN = attn_stage2(b, j, pts.pop((b, j)))
                    last = (b == BLOC - 1 and j == NDC - 1)
                    if last:
                        transposes(b, j, oN, fuse_oproj=True)
                    else:
                        pend_tr = (b, j, oN)
                    if b + 1 < BLOC:
                        prep_slice(b + 1, j)

    nc.compile()
    return nc


def _get_nc(l_chunks=LC_SPARSE // P):
    key = ("nc", l_chunks)
    if key not in _CACHE:
        _CACHE[key] = _build_nc(l_chunks)
    return _CACHE[key]


def _fp8_split(x, e4):
    hi = x.astype(e4)
    lo = (x - hi.astype(np.float32)).astype(e4)
    return hi, lo


def kernel(x_q, x_kv, pad_mask, Wq, Wk, Wv, Wo, bo, rpb):
    from concourse.bass_utils import run_bass_kernel_spmd

    import ml_dtypes

    e4 = ml_dtypes.float8_e4m3
    bf16 = ml_dtypes.bfloat16

    x_q = np.asarray(x_q, dtype=np.float32)
    x_kv = np.asarray(x_kv, dtype=np.float32)
    pad_mask = np.asarray(pad_mask).astype(bool)
    rpb2 = np.asarray(rpb, np.float32).reshape(L, D)

    counts = (~pad_mask).sum(axis=1)
    L_C = LC_SPARSE if counts.max() <= LC_SPARSE else L
    lch = L_C // P
    nc = _get_nc(lch)

    def wsplit(W):
        # [P, 2, NDC, D]: (p, hl, k, e) = split(32 * W[k*128+p, e])
        Wr = np.asarray(W, np.float32).reshape(NDC, P, D).transpose(1, 0, 2)
        hi, lo = _fp8_split(Wr * W_PRESCALE, e4)
        return np.stack([hi, lo], axis=1)

    shared = {
        "Wq8": wsplit(Wq), "Wk8": wsplit(Wk), "Wv8": wsplit(Wv),
        "Wo16": (np.asarray(Wo, np.float32) / W_PRESCALE).astype(bf16),
        "ident": np.eye(P, dtype=bf16),
        "bo": np.asarray(bo, np.float32).reshape(1, D),
    }
    in_maps = []
    for c in range(NCORES):
        xq8 = np.zeros((BLOC, P, 2, NDC, N), e4)
        xk8 = np.zeros((BLOC, P, 2, NDC, L_C), e4)
        rpbT = np.zeros((BLOC, P, NDC, L_C), np.float32)
        mb = np.full((BLOC, L_C), MASK_NEG, np.float32)
        for b in range(BLOC):
            g = c * BLOC + b
            # xq: (p, k, n) = x_q[g][n, k*128+p]
            xqT = x_q[g].T.reshape(NDC, P, N).transpose(1, 0, 2)
            hi, lo = _fp8_split(xqT, e4)
            xq8[b, :, 0], xq8[b, :, 1] = hi, lo
            idx = np.nonzero(~pad_mask[g])[0]
            cnt = len(idx)
            xkT = np.zeros((D, L_C), np.float32)
            xkT[:, :cnt] = x_kv[g, idx, :].T
            xkr = xkT.reshape(NDC, P, L_C).transpose(1, 0, 2)
            hi, lo = _fp8_split(xkr, e4)
            xk8[b, :, 0], xk8[b, :, 1] = hi, lo
            rpbW = np.zeros((D, L_C), np.float32)
            rpbW[:, :cnt] = W_PRESCALE * rpb2[idx, :].T
            rpbT[b] = rpbW.reshape(NDC, P, L_C).transpose(1, 0, 2)
            mb[b, :cnt] = EXP_SHIFT
        in_maps.append({
            "xq8": xq8, "xk8": xk8, "rpbT": rpbT.astype(bf16), "mbias": mb,
            **shared,
        })
    res = run_bass_kernel_spmd(nc, in_maps, list(range(NCORES)))
    return np.concatenate([res.results[c]["out"] for c in range(NCORES)], axis=0)
